# revision 5
# baseline (speedup 1.0000x reference)
"""Trainium2 Bass kernel for a post-LN transformer block (MHA + FFN).

Contract: kernel(**inputs) takes the FULL unsharded inputs (as produced by
the problem's setup_inputs) and returns the FULL output [2, 2048, 1024].

Sharding v2: attention is head-sharded (core c owns heads {2c, 2c+1} for
BOTH batches; QKV weight slices are 128 output columns). This makes
QKV+attention collective-free. The per-head context (1 MB bf16 per core)
is then redistributed with ONE 8-rank mesh AllToAll into token-sharded
layout (core c owns 512 tokens of batch c//4), and Wo/LN1/FFN/LN2 run
token-sharded, entirely in transposed (feature-major) layout.

Matmuls run in bf16 (fp32 PSUM accumulation). LN stats are computed with
ones-vector fp32 matmuls over the partition (feature) axis.
"""
import sys

for _p in ('/opt/trn_rl_repo', '/opt/pypackages'):
    if _p not in sys.path:
        sys.path.insert(0, _p)

import numpy as np
import ml_dtypes
import concourse.bass as bass
import concourse.tile as tile
from concourse import bacc, mybir
from concourse.bass import ts
from contextlib import ExitStack

# ---- profiling shim (enables trace=True under axon; harmless if unused) ----
def _install_prof_shim():
    import types
    if 'antenv.axon_hooks' in sys.modules:
        return
    try:
        import trn_agent_boot.trn_boot as tb
        hook = tb._ntff_profile_via_ctypes('/opt/axon/libaxon_pjrt.so')
    except Exception:
        hook = None
    mod = types.ModuleType('antenv.axon_hooks')
    mod.get_axon_ntff_profile_hook = lambda: hook
    mod.set_axon_ntff_profile_hook = lambda h: None
    sys.modules['antenv.axon_hooks'] = mod

_install_prof_shim()

from concourse.bass_utils import run_bass_kernel_spmd  # noqa: E402

B, S, H, NH, HD = 2, 2048, 1024, 16, 64
P = 128
NCORES = 8
TQ = S // 4                  # tokens per core post-attention = 512
FT = H // P                  # feature tiles = 8
KB = S // P                  # key blocks per batch = 16
QB = S // TQ                 # query blocks per batch = 4
EPS = 1e-5
RG8 = [[0, 1, 2, 3, 4, 5, 6, 7]]
MAGIC = 0x5f3759df + 1

f32 = mybir.dt.float32
bf16 = mybir.dt.bfloat16
i32 = mybir.dt.int32
AF = mybir.ActivationFunctionType
ALU = mybir.AluOpType

W_CHUNK = 256


def build_kernel():
    nc = bacc.Bacc("TRN2", target_bir_lowering=False, debug=False,
                   num_devices=NCORES)

    def din(name, shape, dt=f32):
        return nc.dram_tensor(name, shape, dt, kind="ExternalInput").ap()

    # inputs (per-core values supplied via in_maps)
    xT = din("xT", [H, B * S], bf16)        # full x transposed [feat, tok]
    xres = din("xres", [H, TQ])             # my token slice of x^T + bo (f32)
    wqTs = din("wqTs", [H, P], bf16)        # Wq.T column slice for my 2 heads
    wkTs = din("wkTs", [H, P], bf16)        # Wk.T slice * 0.125
    wvTs = din("wvTs", [H, P], bf16)        # Wv.T slice
    woT = din("woT", [H, H], bf16)
    w1T = din("w1T", [H, H], bf16)
    w2T = din("w2T", [H, H], bf16)
    bqp = din("bqp", [P, 1])                # bq slice as per-partition scalar
    bkp = din("bkp", [P, 1])                # bk slice * 0.125
    bvB = din("bvB", [P, P])                # bv slice broadcast along rows
    b1p = din("b1p", [P, FT])               # [part, tile] layouts
    b2p = din("b2p", [P, FT])
    g1p = din("g1p", [P, FT])
    be1p = din("be1p", [P, FT])
    g2p = din("g2p", [P, FT])
    be2p = din("be2p", [P, FT])
    nri = din("nri", [1, 2], i32)           # [1, -1]
    nrm = din("nrm", [1, TQ], i32)          # rsqrt magic + 1
    y = nc.dram_tensor("y", [H, TQ], f32, kind="ExternalOutput").ap()

    a2a_in = nc.dram_tensor("a2a_in", [NCORES, P, TQ], bf16).ap()
    a2a_out = nc.dram_tensor("a2a_out", [NCORES, P, TQ], bf16).ap()

    with tile.TileContext(nc) as tc, ExitStack() as ctx:
        const = ctx.enter_context(tc.tile_pool(name="const", bufs=1))
        acts = ctx.enter_context(tc.tile_pool(name="acts", bufs=1))
        wpool = ctx.enter_context(tc.tile_pool(name="w", bufs=3))

        # constants
        bq_s = const.tile([P, 1], f32)
        nc.sync.dma_start(bq_s[:], bqp)
        bk_s = const.tile([P, 1], f32)
        nc.sync.dma_start(bk_s[:], bkp)
        bvB_s = const.tile([P, P], f32)
        nc.sync.dma_start(bvB_s[:], bvB)
        b1_s = const.tile([P, FT], f32)
        nc.sync.dma_start(b1_s[:], b1p)
        b2_s = const.tile([P, FT], f32)
        nc.sync.dma_start(b2_s[:], b2p)
        g1_s = const.tile([P, FT], f32)
        nc.sync.dma_start(g1_s[:], g1p)
        be1_s = const.tile([P, FT], f32)
        nc.sync.dma_start(be1_s[:], be1p)
        g2_s = const.tile([P, FT], f32)
        nc.sync.dma_start(g2_s[:], g2p)
        be2_s = const.tile([P, FT], f32)
        nc.sync.dma_start(be2_s[:], be2p)
        nri_s = const.tile([1, 2], i32)
        nc.sync.dma_start(nri_s[:], nri)
        nrm_s = const.tile([1, TQ], i32)
        nc.sync.dma_start(nrm_s[:], nrm)
        ones_f = const.tile([P, 1], f32)
        nc.vector.memset(ones_f[:], 1.0)
        eps_s = const.tile([P, 1], f32)
        nc.vector.memset(eps_s[:], EPS)
        warm_s = const.tile([P, 1], f32)
        nc.scalar.activation(warm_s[:], eps_s[:], AF.Exp)  # load exp table

        # persistent activations
        qt_s = acts.tile([P, B, S], bf16)        # Q^T (2 heads on partitions)
        kt_s = acts.tile([P, B, S], bf16)        # K^T (pre-scaled)
        v_s = acts.tile([P, B, KB, 2, HD + 1], bf16)  # V + ones col per head
        ctxT_s = acts.tile([P, B, S], bf16)      # normalized ctx^T (my heads)
        xres_s = acts.tile([P, FT, TQ], f32)     # my tokens x^T + bo
        ctxF_s = acts.tile([P, FT, TQ], bf16)    # full ctx^T for my tokens
        ln1f_s = acts.tile([P, FT, TQ], f32)     # LN1 out f32 (residual)
        ln1b_s = acts.tile([P, FT, TQ], bf16)    # LN1 out bf16 (fc1 input)
        hT_s = acts.tile([P, FT, TQ], bf16)      # relu(fc1)

        nc.gpsimd.dma_start(xres_s[:], xres.rearrange("(t p) n -> p t n", p=P))

        # ones columns for softmax denominators
        for b in range(B):
            nc.vector.memset(v_s[:, b, :, :, HD:HD + 1], 1.0)

        # ---------------- phase A: QKV projections ----------------
        with tc.tile_pool(name="xp", bufs=1) as xp, \
             tc.tile_pool(name="psA", bufs=2, space="PSUM") as psA, \
             tc.tile_pool(name="psV", bufs=2, space="PSUM") as psV:
            wq_s = xp.tile([P, FT, P], bf16)
            nc.sync.dma_start(wq_s[:], wqTs.rearrange("(t p) m -> p t m", p=P))
            wk_s = xp.tile([P, FT, P], bf16)
            nc.sync.dma_start(wk_s[:], wkTs.rearrange("(t p) m -> p t m", p=P))
            wv_s = xp.tile([P, FT, P], bf16)
            nc.sync.dma_start(wv_s[:], wvTs.rearrange("(t p) m -> p t m", p=P))
            xt = xp.tile([P, B, FT, S], bf16)
            xt_r = xT.rearrange("(t p) (b n) -> p b t n", p=P, b=B)
            for b in range(B):
                for kt in range(FT):
                    nc.sync.dma_start(xt[:, b, kt, :], xt_r[:, b, kt, :])
            for b in range(B):
                for qtr in range(4):           # Q^T, K^T in N=512 chunks
                    ps = psA.tile([P, TQ], f32, tag="psA")
                    for kt in range(FT):
                        nc.tensor.matmul(ps[:], wq_s[:, kt, :],
                                         xt[:, b, kt, ts(qtr, TQ)],
                                         start=(kt == 0), stop=(kt == FT - 1))
                    nc.vector.tensor_scalar(
                        out=qt_s[:, b, ts(qtr, TQ)], in0=ps[:],
                        scalar1=bq_s[:, 0:1], scalar2=None, op0=ALU.add)
                    ps = psA.tile([P, TQ], f32, tag="psA")
                    for kt in range(FT):
                        nc.tensor.matmul(ps[:], wk_s[:, kt, :],
                                         xt[:, b, kt, ts(qtr, TQ)],
                                         start=(kt == 0), stop=(kt == FT - 1))
                    nc.vector.tensor_scalar(
                        out=kt_s[:, b, ts(qtr, TQ)], in0=ps[:],
                        scalar1=bk_s[:, 0:1], scalar2=None, op0=ALU.add)
                for tb in range(KB):           # V natural [tok, feat]
                    ps = psV.tile([P, P], f32, tag="psV")
                    for kt in range(FT):
                        nc.tensor.matmul(ps[:], xt[:, b, kt, ts(tb, P)],
                                         wv_s[:, kt, :],
                                         start=(kt == 0), stop=(kt == FT - 1))
                    nc.vector.tensor_tensor(
                        out=v_s[:, b, tb, :, 0:HD], in0=ps.rearrange("p (h d) -> p h d", h=2),
                        in1=bvB_s.rearrange("p (h d) -> p h d", h=2), op=ALU.add)

        # ---------------- phase B: attention ----------------
        with tc.tile_pool(name="psS", bufs=2, space="PSUM") as psS, \
             tc.tile_pool(name="psC", bufs=2, space="PSUM") as psC, \
             tc.tile_pool(name="esb", bufs=4) as esb, \
             tc.tile_pool(name="rec", bufs=2) as rec:
            for b in range(B):
                for qb in range(QB):
                    ps_c0 = psC.tile([P, TQ], f32, tag="c0")
                    ps_c1 = psC.tile([P, TQ], f32, tag="c1")
                    for kb in range(KB):
                        last = (kb == KB - 1)
                        ps = psS.tile([P, 2, TQ], f32, tag="s")
                        nc.tensor.matmul(ps[:, 0, :],
                                         kt_s[0:HD, b, ts(kb, P)],
                                         qt_s[0:HD, b, ts(qb, TQ)],
                                         start=True, stop=True)
                        nc.tensor.matmul(ps[:, 1, :],
                                         kt_s[HD:P, b, ts(kb, P)],
                                         qt_s[HD:P, b, ts(qb, TQ)],
                                         start=True, stop=True)
                        e = esb.tile([P, 2, TQ], bf16, tag="e")
                        nc.scalar.activation(e[:], ps[:], AF.Exp)
                        nc.tensor.matmul(ps_c0[0:HD + 1, :], v_s[:, b, kb, 0, :],
                                         e[:, 0, :], start=(kb == 0), stop=last)
                        nc.tensor.matmul(ps_c1[0:HD + 1, :], v_s[:, b, kb, 1, :],
                                         e[:, 1, :], start=(kb == 0), stop=last)
                    # normalize rows 0-63 by row 64
                    sr0 = rec.tile([HD + 1, TQ], f32, tag="sr0")
                    nc.vector.tensor_copy(sr0[HD:HD + 1, :], ps_c0[HD:HD + 1, :])
                    rr0 = rec.tile([1, TQ], f32, tag="rr0")
                    nc.gpsimd.dma_start(rr0[:], sr0[HD:HD + 1, :])
                    nc.vector.reciprocal_approx_fast(rr0[:], rr0[:])
                    rb0 = rec.tile([HD, TQ], f32, tag="rb0")
                    nc.gpsimd.partition_broadcast(rb0[:], rr0[:])
                    nc.vector.tensor_tensor(
                        out=ctxT_s[0:HD, b, ts(qb, TQ)], in0=ps_c0[0:HD, :],
                        in1=rb0[:], op=ALU.mult)
                    sr1 = rec.tile([HD + 1, TQ], f32, tag="sr1")
                    nc.vector.tensor_copy(sr1[HD:HD + 1, :], ps_c1[HD:HD + 1, :])
                    rr1 = rec.tile([1, TQ], f32, tag="rr1")
                    nc.gpsimd.dma_start(rr1[:], sr1[HD:HD + 1, :])
                    nc.vector.reciprocal_approx_fast(rr1[:], rr1[:])
                    rb1 = rec.tile([HD, TQ], f32, tag="rb1")
                    nc.gpsimd.partition_broadcast(rb1[:], rr1[:])
                    c1t = rec.tile([HD, TQ], bf16, tag="c1t")
                    nc.vector.tensor_tensor(out=c1t[:], in0=ps_c1[0:HD, :],
                                            in1=rb1[:], op=ALU.mult)
                    nc.gpsimd.dma_start(ctxT_s[HD:P, b, ts(qb, TQ)], c1t[:])

        # ---------------- AllToAll: ctx -> token-sharded ----------------
        for j in range(NCORES):
            nc.sync.dma_start(a2a_in[j], ctxT_s[:, j // 4, ts(j % 4, TQ)])
        nc.gpsimd.collective_compute(
            "AllToAll", ALU.bypass, replica_groups=RG8,
            ins=[a2a_in], outs=[a2a_out])
        nc.sync.dma_start(ctxF_s[:], a2a_out.rearrange("r p n -> p r n"))

        # ---------------- transposed projection helper ----------------
        def proj_T(wap, kxmT_s, evict, psA):
            for half in range(H // W_CHUNK):
                w_s = wpool.tile([P, FT, W_CHUNK], bf16, tag="w")
                nc.sync.dma_start(
                    w_s[:],
                    wap.rearrange("(t p) m -> p t m", p=P)[:, :, ts(half, W_CHUNK)])
                for mi in range(W_CHUNK // P):
                    mt_i = half * (W_CHUNK // P) + mi
                    ps = psA.tile([P, TQ], f32, tag="psA")
                    for kt in range(FT):
                        nc.tensor.matmul(ps[:], w_s[:, kt, ts(mi, P)],
                                         kxmT_s[:, kt, :],
                                         start=(kt == 0), stop=(kt == FT - 1))
                    evict(mt_i, ps)

        # ---------------- transposed LN helper ----------------
        def layernorm_T(src_s, g_s, be_s, out_f, out_b, lnp, psT, psSt):
            # src_s: [P, FT, TQ] f32; stats over partition axis x 8 tiles
            st0 = psSt.tile([1, TQ], f32, tag="st0")
            st1 = psSt.tile([1, TQ], f32, tag="st1")
            for ft in range(FT):
                sq = lnp.tile([P, TQ], f32, tag="sq")
                nc.vector.tensor_tensor(out=sq[:], in0=src_s[:, ft, :],
                                        in1=src_s[:, ft, :], op=ALU.mult)
                nc.tensor.matmul(st0[:], ones_f[:], src_s[:, ft, :],
                                 start=(ft == 0), stop=(ft == FT - 1))
                nc.tensor.matmul(st1[:], ones_f[:], sq[:],
                                 start=(ft == 0), stop=(ft == FT - 1))
            mu = lnp.tile([1, TQ], f32, tag="mu")
            nc.vector.tensor_scalar(out=mu[:], in0=st0[:],
                                    scalar1=1.0 / H, scalar2=None, op0=ALU.mult)
            ve = lnp.tile([1, TQ], f32, tag="ve")
            nc.vector.tensor_scalar(out=ve[:], in0=st1[:],
                                    scalar1=1.0 / H, scalar2=None, op0=ALU.mult)
            mu2 = lnp.tile([1, TQ], f32, tag="mu2")
            nc.vector.tensor_tensor(out=mu2[:], in0=mu[:], in1=mu[:],
                                    op=ALU.mult)
            nc.vector.tensor_tensor(out=ve[:], in0=ve[:], in1=mu2[:],
                                    op=ALU.subtract)
            nc.vector.tensor_scalar(out=ve[:], in0=ve[:], scalar1=EPS,
                                    scalar2=None, op0=ALU.add)
            # rstd via int magic + 2 Newton steps
            it = lnp.tile([1, TQ], i32, tag="it")
            nc.vector.tensor_scalar(out=it[:], in0=ve.bitcast(i32),
                                    scalar1=nri_s[:, 0:1], scalar2=None,
                                    op0=ALU.logical_shift_right)
            nc.vector.tensor_scalar(out=it[:], in0=it[:], scalar1=nri_s[:, 1:2],
                                    scalar2=None, op0=ALU.bitwise_xor)
            nc.vector.tensor_tensor(out=it[:], in0=it[:], in1=nrm_s[:],
                                    op=ALU.add)
            rstd = it.bitcast(f32)
            nrt = lnp.tile([1, TQ], f32, tag="nrt")
            for _ in range(2):
                nc.vector.tensor_tensor(out=nrt[:], in0=rstd, in1=rstd,
                                        op=ALU.mult)
                nc.vector.tensor_tensor(out=nrt[:], in0=nrt[:], in1=ve[:],
                                        op=ALU.mult)
                nc.vector.tensor_scalar(out=nrt[:], in0=nrt[:], scalar1=-0.5,
                                        scalar2=1.5, op0=ALU.mult, op1=ALU.add)
                nc.vector.tensor_tensor(out=rstd, in0=rstd, in1=nrt[:],
                                        op=ALU.mult)
            muB = lnp.tile([P, TQ], f32, tag="muB")
            nc.gpsimd.partition_broadcast(muB[:], mu[:])
            rsB = lnp.tile([P, TQ], f32, tag="rsB")
            nc.gpsimd.partition_broadcast(rsB[:], rstd)
            for ft in range(FT):
                z = lnp.tile([P, TQ], f32, tag="z")
                nc.vector.tensor_tensor(out=z[:], in0=src_s[:, ft, :],
                                        in1=muB[:], op=ALU.subtract)
                nc.vector.tensor_tensor(out=z[:], in0=z[:], in1=rsB[:],
                                        op=ALU.mult)
                nc.vector.tensor_scalar(
                    out=out_f[:, ft, :], in0=z[:],
                    scalar1=g_s[:, ft:ft + 1], scalar2=be_s[:, ft:ft + 1],
                    op0=ALU.mult, op1=ALU.add)
                if out_b is not None:
                    nc.scalar.copy(out_b[:, ft, :], out_f[:, ft, :])

        # ---------------- phases C-F (all feature-major) ----------------
        with tc.tile_pool(name="lnp", bufs=3) as lnp, \
             tc.tile_pool(name="psA2", bufs=2, space="PSUM") as psA2, \
             tc.tile_pool(name="psSt", bufs=1, space="PSUM") as psSt:
            t1_s = acts.tile([P, FT, TQ], f32, tag="tres")

            def wo_evict(mt_i, ps):
                nc.vector.tensor_tensor(out=t1_s[:, mt_i, :], in0=ps[:],
                                        in1=xres_s[:, mt_i, :], op=ALU.add)

            proj_T(woT, ctxF_s, wo_evict, psA2)
            layernorm_T(t1_s, g1_s, be1_s, ln1f_s, ln1b_s, lnp, psA2, psSt)

            def relu_evict(mt_i, ps):
                nc.vector.tensor_scalar(
                    out=hT_s[:, mt_i, :], in0=ps[:],
                    scalar1=b1_s[:, mt_i:mt_i + 1], scalar2=0.0,
                    op0=ALU.add, op1=ALU.max)

            proj_T(w1T, ln1b_s, relu_evict, psA2)

            t2_s = acts.tile([P, FT, TQ], f32, tag="tres")

            def w2_evict(mt_i, ps):
                z = lnp.tile([P, TQ], f32, tag="w2z")
                nc.vector.tensor_scalar(
                    out=z[:], in0=ps[:],
                    scalar1=b2_s[:, mt_i:mt_i + 1], scalar2=None, op0=ALU.add)
                nc.vector.tensor_tensor(out=t2_s[:, mt_i, :], in0=z[:],
                                        in1=ln1f_s[:, mt_i, :], op=ALU.add)

            proj_T(w2T, hT_s, w2_evict, psA2)

            yT_s = acts.tile([P, FT, TQ], f32, tag="yt")
            layernorm_T(t2_s, g2_s, be2_s, yT_s, None, lnp, psA2, psSt)
            nc.sync.dma_start(y.rearrange("(t p) n -> p t n", p=P), yT_s[:])

    nc.compile()
    return nc


_NC_CACHE = {}


def _get_nc():
    if 'nc' not in _NC_CACHE:
        _NC_CACHE['nc'] = build_kernel()
    return _NC_CACHE['nc']


def _bf(a):
    return np.ascontiguousarray(np.asarray(a, np.float32)).astype(
        ml_dtypes.bfloat16)


def make_in_maps(x, Wq, bq, Wk, bk, Wv, bv, Wo, bo, W1, b1, W2, b2,
                 g1, be1, g2, be2):
    def pt(v):  # [H] -> [P, FT] partition-tiled
        return np.ascontiguousarray(np.asarray(v, np.float32).reshape(FT, P).T)

    scale = np.float32(1.0 / np.sqrt(HD))
    x = np.asarray(x, np.float32)
    xTf = np.ascontiguousarray(x.transpose(2, 0, 1).reshape(H, B * S))
    wqT = np.asarray(Wq, np.float32).T
    wkT = np.asarray(Wk, np.float32).T * scale
    wvT = np.asarray(Wv, np.float32).T
    bo = np.asarray(bo, np.float32)
    shared = {
        "xT": _bf(xTf),
        "woT": _bf(np.asarray(Wo, np.float32).T),
        "w1T": _bf(np.asarray(W1, np.float32).T),
        "w2T": _bf(np.asarray(W2, np.float32).T),
        "b1p": pt(b1), "b2p": pt(b2),
        "g1p": pt(g1), "be1p": pt(be1), "g2p": pt(g2), "be2p": pt(be2),
        "nri": np.array([[1, -1]], np.int32),
        "nrm": np.full((1, TQ), MAGIC, np.int32),
    }
    in_maps = []
    for c in range(NCORES):
        hs = slice(P * c, P * (c + 1))
        b, sl = c // 4, (c % 4) * TQ
        m = dict(shared)
        m["wqTs"] = _bf(wqT[:, hs])
        m["wkTs"] = _bf(wkT[:, hs])
        m["wvTs"] = _bf(wvT[:, hs])
        m["bqp"] = np.ascontiguousarray(
            np.asarray(bq, np.float32)[hs].reshape(P, 1))
        m["bkp"] = np.ascontiguousarray(
            (np.asarray(bk, np.float32) * scale)[hs].reshape(P, 1))
        m["bvB"] = np.ascontiguousarray(np.broadcast_to(
            np.asarray(bv, np.float32)[hs], (P, P)))
        m["xres"] = np.ascontiguousarray(x[b, sl:sl + TQ, :].T + bo[:, None])
        in_maps.append(m)
    return in_maps


def kernel(x, Wq, bq, Wk, bk, Wv, bv, Wo, bo, W1, b1, W2, b2,
           g1, be1, g2, be2):
    x = np.asarray(x)
    nc = _get_nc()
    in_maps = make_in_maps(x, Wq, bq, Wk, bk, Wv, bv, Wo, bo,
                           W1, b1, W2, b2, g1, be1, g2, be2)
    res = run_bass_kernel_spmd(nc, in_maps, list(range(NCORES)))
    out = np.empty((B, S, H), np.float32)
    for c in range(NCORES):
        b, sl = c // 4, (c % 4) * TQ
        out[b, sl:sl + TQ, :] = np.asarray(res.results[c]["y"]).T
    return out


# revision 8
# speedup vs baseline: 1.0280x; 1.0280x over previous
"""Trainium2 Bass kernel for a post-LN transformer block (MHA + FFN).

Contract: kernel(**inputs) takes the FULL unsharded inputs (as produced by
the problem's setup_inputs) and returns the FULL output [2, 2048, 1024].

Sharding v3: attention is head-sharded (core c owns heads {2c, 2c+1} for
BOTH batches). QKV+attention are collective-free; batch-1 QKV work is
emitted interleaved with batch-0 attention units so it hides under the
exp-bound attention window. The per-head context (1 MB bf16 per core) is
redistributed with ONE 8-rank mesh AllToAll into token-sharded layout
(core c owns 512 tokens of batch c//4); Wo/LN1/FFN/LN2 then run
token-sharded in transposed (feature-major) layout. LN stats use bf16
ones-vector matmuls over the partition (feature) axis.
"""
import sys

for _p in ('/opt/trn_rl_repo', '/opt/pypackages'):
    if _p not in sys.path:
        sys.path.insert(0, _p)

import numpy as np
import ml_dtypes
import concourse.bass as bass
import concourse.tile as tile
from concourse import bacc, mybir
from concourse.bass import ts
from contextlib import ExitStack

# ---- profiling shim (enables trace=True under axon; harmless if unused) ----
def _install_prof_shim():
    import types
    if 'antenv.axon_hooks' in sys.modules:
        return
    try:
        import trn_agent_boot.trn_boot as tb
        hook = tb._ntff_profile_via_ctypes('/opt/axon/libaxon_pjrt.so')
    except Exception:
        hook = None
    mod = types.ModuleType('antenv.axon_hooks')
    mod.get_axon_ntff_profile_hook = lambda: hook
    mod.set_axon_ntff_profile_hook = lambda h: None
    sys.modules['antenv.axon_hooks'] = mod

_install_prof_shim()

from concourse.bass_utils import run_bass_kernel_spmd  # noqa: E402

B, S, H, NH, HD = 2, 2048, 1024, 16, 64
P = 128
NCORES = 8
TQ = S // 4                  # tokens per core post-attention = 512
FT = H // P                  # feature tiles = 8
KB = S // P                  # key blocks per batch = 16
QB = S // TQ                 # query blocks per batch = 4
EPS = 1e-5
RG8 = [[0, 1, 2, 3, 4, 5, 6, 7]]
MAGIC = 0x5f3759df + 1

f32 = mybir.dt.float32
bf16 = mybir.dt.bfloat16
i32 = mybir.dt.int32
AF = mybir.ActivationFunctionType
ALU = mybir.AluOpType

W_CHUNK = 256


def build_kernel():
    nc = bacc.Bacc("TRN2", target_bir_lowering=False, debug=False,
                   num_devices=NCORES)

    def din(name, shape, dt=f32):
        return nc.dram_tensor(name, shape, dt, kind="ExternalInput").ap()

    xT = din("xT", [H, B * S], bf16)        # full x transposed [feat, tok]
    xres = din("xres", [H, TQ])             # my token slice of x^T + bo (f32)
    wqTs = din("wqTs", [H, P], bf16)        # Wq.T column slice for my 2 heads
    wkTs = din("wkTs", [H, P], bf16)        # Wk.T slice * 0.125
    wvTs = din("wvTs", [H, P], bf16)        # Wv.T slice
    woT = din("woT", [H, H], bf16)
    w1T = din("w1T", [H, H], bf16)
    w2T = din("w2T", [H, H], bf16)
    bqp = din("bqp", [P, 1])
    bkp = din("bkp", [P, 1])
    bvB = din("bvB", [P, P])
    b1p = din("b1p", [P, FT])
    b2p = din("b2p", [P, FT])
    g1p = din("g1p", [P, FT])
    be1p = din("be1p", [P, FT])
    g2p = din("g2p", [P, FT])
    be2p = din("be2p", [P, FT])
    nri = din("nri", [1, 2], i32)
    nrm = din("nrm", [1, TQ], i32)
    y = nc.dram_tensor("y", [H, TQ], f32, kind="ExternalOutput").ap()

    a2a_in = nc.dram_tensor("a2a_in", [NCORES, P, TQ], bf16).ap()
    a2a_out = nc.dram_tensor("a2a_out", [NCORES, P, TQ], bf16).ap()

    with tile.TileContext(nc) as tc, ExitStack() as ctx:
        const = ctx.enter_context(tc.tile_pool(name="const", bufs=1))
        acts = ctx.enter_context(tc.tile_pool(name="acts", bufs=1))
        wpool = ctx.enter_context(tc.tile_pool(name="w", bufs=3))

        # constants
        bq_s = const.tile([P, 1], f32)
        nc.sync.dma_start(bq_s[:], bqp)
        bk_s = const.tile([P, 1], f32)
        nc.sync.dma_start(bk_s[:], bkp)
        bvB_s = const.tile([P, P], f32)
        nc.sync.dma_start(bvB_s[:], bvB)
        b1_s = const.tile([P, FT], f32)
        nc.sync.dma_start(b1_s[:], b1p)
        b2_s = const.tile([P, FT], f32)
        nc.sync.dma_start(b2_s[:], b2p)
        g1_s = const.tile([P, FT], f32)
        nc.sync.dma_start(g1_s[:], g1p)
        be1_s = const.tile([P, FT], f32)
        nc.sync.dma_start(be1_s[:], be1p)
        g2_s = const.tile([P, FT], f32)
        nc.sync.dma_start(g2_s[:], g2p)
        be2_s = const.tile([P, FT], f32)
        nc.sync.dma_start(be2_s[:], be2p)
        nri_s = const.tile([1, 2], i32)
        nc.sync.dma_start(nri_s[:], nri)
        nrm_s = const.tile([1, TQ], i32)
        nc.sync.dma_start(nrm_s[:], nrm)
        ones_b = const.tile([P, 1], bf16)
        nc.vector.memset(ones_b[:], 1.0)
        eps_s = const.tile([P, 1], f32)
        nc.vector.memset(eps_s[:], EPS)
        warm_s = const.tile([P, 1], f32)
        nc.scalar.activation(warm_s[:], eps_s[:], AF.Exp)  # load exp table

        # persistent activations
        qt_s = acts.tile([P, B, S], bf16)
        kt_s = acts.tile([P, B, S], bf16)
        v_s = acts.tile([P, B, KB, 2, HD + 1], bf16)
        ctxT_s = acts.tile([P, B, S], bf16)
        xres_s = acts.tile([P, FT, TQ], f32)
        ctxF_s = acts.tile([P, FT, TQ], bf16)
        ln1b_s = acts.tile([P, FT, TQ], bf16)
        hT_s = acts.tile([P, FT, TQ], bf16)

        # weight + x tiles (per-(b,kt) tiles so deps are fine-grained)
        xpctx = ExitStack()
        xp = xpctx.enter_context(tc.tile_pool(name="xp", bufs=1))
        wq_s = xp.tile([P, FT, P], bf16)
        nc.sync.dma_start(wq_s[:], wqTs.rearrange("(t p) m -> p t m", p=P))
        wk_s = xp.tile([P, FT, P], bf16)
        nc.sync.dma_start(wk_s[:], wkTs.rearrange("(t p) m -> p t m", p=P))
        wv_s = xp.tile([P, FT, P], bf16)
        nc.sync.dma_start(wv_s[:], wvTs.rearrange("(t p) m -> p t m", p=P))
        xt_r = xT.rearrange("(t p) (b n) -> p b t n", p=P, b=B)
        xt = [[None] * FT for _ in range(B)]
        for b in range(B):
            for kt in range(FT):
                xt[b][kt] = xp.tile([P, S], bf16, tag=f"xt{b}_{kt}",
                                    name=f"xt{b}_{kt}")
                nc.sync.dma_start(xt[b][kt][:], xt_r[:, b, kt, :])
        nc.gpsimd.dma_start(xres_s[:], xres.rearrange("(t p) n -> p t n", p=P))
        for b in range(B):
            nc.vector.memset(v_s[:, b, :, :, HD:HD + 1], 1.0)

        # ---------------- batch-0 QKV (kt-outer for early start) ----------
        with tc.tile_pool(name="psQK", bufs=1, space="PSUM") as psQK:
            pq = psQK.tile([P, QB, TQ], f32, tag="pq")
            pk = psQK.tile([P, QB, TQ], f32, tag="pk")
            for kt in range(FT):
                for c in range(QB):
                    nc.tensor.matmul(pq[:, c, :], wq_s[:, kt, :],
                                     xt[0][kt][:, ts(c, TQ)],
                                     start=(kt == 0), stop=(kt == FT - 1))
                    nc.tensor.matmul(pk[:, c, :], wk_s[:, kt, :],
                                     xt[0][kt][:, ts(c, TQ)],
                                     start=(kt == 0), stop=(kt == FT - 1))
            for c in range(QB):
                nc.vector.tensor_scalar(
                    out=qt_s[:, 0, ts(c, TQ)], in0=pq[:, c, :],
                    scalar1=bq_s[:, 0:1], scalar2=None, op0=ALU.add)
                nc.vector.tensor_scalar(
                    out=kt_s[:, 0, ts(c, TQ)], in0=pk[:, c, :],
                    scalar1=bk_s[:, 0:1], scalar2=None, op0=ALU.add)

        def v_block(b, tb, psV):
            ps = psV.tile([P, P], f32, tag="psV")
            for kt in range(FT):
                nc.tensor.matmul(ps[:], xt[b][kt][:, ts(tb, P)],
                                 wv_s[:, kt, :],
                                 start=(kt == 0), stop=(kt == FT - 1))
            nc.vector.tensor_tensor(
                out=v_s[:, b, tb, :, 0:HD],
                in0=ps.rearrange("p (h d) -> p h d", h=2),
                in1=bvB_s.rearrange("p (h d) -> p h d", h=2), op=ALU.add)

        def qk_chunk(b, c, psA):
            ps = psA.tile([P, TQ], f32, tag="psA")
            for kt in range(FT):
                nc.tensor.matmul(ps[:], wq_s[:, kt, :],
                                 xt[b][kt][:, ts(c, TQ)],
                                 start=(kt == 0), stop=(kt == FT - 1))
            nc.vector.tensor_scalar(
                out=qt_s[:, b, ts(c, TQ)], in0=ps[:],
                scalar1=bq_s[:, 0:1], scalar2=None, op0=ALU.add)
            ps = psA.tile([P, TQ], f32, tag="psA")
            for kt in range(FT):
                nc.tensor.matmul(ps[:], wk_s[:, kt, :],
                                 xt[b][kt][:, ts(c, TQ)],
                                 start=(kt == 0), stop=(kt == FT - 1))
            nc.vector.tensor_scalar(
                out=kt_s[:, b, ts(c, TQ)], in0=ps[:],
                scalar1=bk_s[:, 0:1], scalar2=None, op0=ALU.add)

        # batch-0 V (own pool scope, before attention claims PSUM)
        with tc.tile_pool(name="psV0", bufs=2, space="PSUM") as psV0:
            for tb in range(KB):
                v_block(0, tb, psV0)

        # ---------------- attention (+ batch-1 QKV interleaved) -----------
        with tc.tile_pool(name="psS", bufs=2, space="PSUM") as psS, \
             tc.tile_pool(name="psC", bufs=1, space="PSUM") as psC, \
             tc.tile_pool(name="psB1", bufs=1, space="PSUM") as psB1, \
             tc.tile_pool(name="esb", bufs=3) as esb, \
             tc.tile_pool(name="rec", bufs=1) as rec:

            def attn_unit(b, qb):
                ps_c0 = psC.tile([P, TQ], f32, tag="c0")
                ps_c1 = psC.tile([P, TQ], f32, tag="c1")
                for kb in range(KB):
                    last = (kb == KB - 1)
                    ps = psS.tile([P, 2, TQ], f32, tag="s")
                    nc.tensor.matmul(ps[:, 0, :], kt_s[0:HD, b, ts(kb, P)],
                                     qt_s[0:HD, b, ts(qb, TQ)],
                                     start=True, stop=True)
                    nc.tensor.matmul(ps[:, 1, :], kt_s[HD:P, b, ts(kb, P)],
                                     qt_s[HD:P, b, ts(qb, TQ)],
                                     start=True, stop=True)
                    e = esb.tile([P, 2, TQ], bf16, tag="e")
                    nc.scalar.activation(e[:], ps[:], AF.Exp)
                    nc.tensor.matmul(ps_c0[0:HD + 1, :], v_s[:, b, kb, 0, :],
                                     e[:, 0, :], start=(kb == 0), stop=last)
                    nc.tensor.matmul(ps_c1[0:HD + 1, :], v_s[:, b, kb, 1, :],
                                     e[:, 1, :], start=(kb == 0), stop=last)
                # normalize rows 0-63 by row 64
                sr0 = rec.tile([HD + 1, TQ], f32, tag="sr0")
                nc.vector.tensor_copy(sr0[HD:HD + 1, :], ps_c0[HD:HD + 1, :])
                rr0 = rec.tile([1, TQ], f32, tag="rr0")
                nc.gpsimd.dma_start(rr0[:], sr0[HD:HD + 1, :])
                nc.vector.reciprocal_approx_fast(rr0[:], rr0[:])
                rb0 = rec.tile([HD, TQ], f32, tag="rb0")
                nc.gpsimd.partition_broadcast(rb0[:], rr0[:])
                nc.vector.tensor_tensor(
                    out=ctxT_s[0:HD, b, ts(qb, TQ)], in0=ps_c0[0:HD, :],
                    in1=rb0[:], op=ALU.mult)
                sr1 = rec.tile([HD + 1, TQ], f32, tag="sr1")
                nc.vector.tensor_copy(sr1[HD:HD + 1, :], ps_c1[HD:HD + 1, :])
                rr1 = rec.tile([1, TQ], f32, tag="rr1")
                nc.gpsimd.dma_start(rr1[:], sr1[HD:HD + 1, :])
                nc.vector.reciprocal_approx_fast(rr1[:], rr1[:])
                rb1 = rec.tile([HD, TQ], f32, tag="rb1")
                nc.gpsimd.partition_broadcast(rb1[:], rr1[:])
                c1t = rec.tile([HD, TQ], bf16, tag="c1t")
                nc.vector.tensor_tensor(out=c1t[:], in0=ps_c1[0:HD, :],
                                        in1=rb1[:], op=ALU.mult)
                nc.gpsimd.dma_start(ctxT_s[HD:P, b, ts(qb, TQ)], c1t[:])
                j = b * QB + qb
                nc.sync.dma_start(a2a_in[j], ctxT_s[:, b, ts(qb, TQ)])

            for qb in range(QB):          # batch-0 units + b1 QKV slices
                attn_unit(0, qb)
                qk_chunk(1, qb, psB1)
                for tb in range(4 * qb, 4 * qb + 4):
                    v_block(1, tb, psB1)
            for qb in range(QB):
                attn_unit(1, qb)
        xpctx.close()

        # ---------------- AllToAll: ctx -> token-sharded ----------------
        # prefetch first Wo chunk so its DMA fills the collective wait
        w_pre = wpool.tile([P, FT, W_CHUNK], bf16, tag="w")
        nc.sync.dma_start(
            w_pre[:], woT.rearrange("(t p) m -> p t m", p=P)[:, :, 0:W_CHUNK])
        nc.gpsimd.collective_compute(
            "AllToAll", ALU.bypass, replica_groups=RG8,
            ins=[a2a_in], outs=[a2a_out])
        nc.sync.dma_start(ctxF_s[:], a2a_out.rearrange("r p n -> p r n"))

        # ---------------- transposed projection helper ----------------
        def proj_T(wap, kxmT_s, evict, psA, w_first=None):
            for half in range(H // W_CHUNK):
                if half == 0 and w_first is not None:
                    w_s = w_first
                else:
                    w_s = wpool.tile([P, FT, W_CHUNK], bf16, tag="w")
                    nc.sync.dma_start(
                        w_s[:],
                        wap.rearrange("(t p) m -> p t m", p=P)[:, :, ts(half, W_CHUNK)])
                for mi in range(W_CHUNK // P):
                    mt_i = half * (W_CHUNK // P) + mi
                    ps = psA.tile([P, TQ], f32, tag="psA")
                    for kt in range(FT):
                        nc.tensor.matmul(ps[:], w_s[:, kt, ts(mi, P)],
                                         kxmT_s[:, kt, :],
                                         start=(kt == 0), stop=(kt == FT - 1))
                    evict(mt_i, ps)

        # ------------- transposed LN (stats pipelined via callbacks) ------
        def make_stats(lnp, psSt, name):
            st0 = psSt.tile([1, TQ], f32, tag=f"{name}0")
            st1 = psSt.tile([1, TQ], f32, tag=f"{name}1")
            pend = []

            def emit_one():
                mt_j, tbj, sqj = pend.pop(0)
                nc.tensor.matmul(st0[:], ones_b[:], tbj[:],
                                 start=(mt_j == 0), stop=(mt_j == FT - 1))
                nc.tensor.matmul(st1[:], ones_b[:], sqj[:],
                                 start=(mt_j == 0), stop=(mt_j == FT - 1))

            def feed(mt_i, tb):
                # tb: [P, TQ] bf16 tile of the pre-LN value
                sq = lnp.tile([P, TQ], bf16, tag=f"sq{mt_i % 3}")
                nc.vector.tensor_tensor(out=sq[:], in0=tb[:], in1=tb[:],
                                        op=ALU.mult)
                pend.append((mt_i, tb, sq))
                if len(pend) > 1:       # delayed emission: keep tensor busy
                    emit_one()

            def flush():
                while pend:
                    emit_one()

            return st0, st1, feed, flush

        def ln_apply(src_f, st0, st1, g_s, be_s, lnp, out_cb):
            # src_f: [P, FT, TQ] f32. out_cb(ft, z, g_s, be_s) consumes z.
            mu = lnp.tile([1, TQ], f32, tag="mu")
            nc.vector.tensor_scalar(out=mu[:], in0=st0[:], scalar1=1.0 / H,
                                    scalar2=None, op0=ALU.mult)
            muB = lnp.tile([P, TQ], f32, tag="muB")
            nc.gpsimd.partition_broadcast(muB[:], mu[:])
            ve = lnp.tile([1, TQ], f32, tag="ve")
            nc.vector.tensor_scalar(out=ve[:], in0=st1[:], scalar1=1.0 / H,
                                    scalar2=None, op0=ALU.mult)
            mu2 = lnp.tile([1, TQ], f32, tag="mu2")
            nc.vector.tensor_tensor(out=mu2[:], in0=mu[:], in1=mu[:],
                                    op=ALU.mult)
            nc.vector.tensor_tensor(out=ve[:], in0=ve[:], in1=mu2[:],
                                    op=ALU.subtract)
            nc.vector.tensor_scalar(out=ve[:], in0=ve[:], scalar1=EPS,
                                    scalar2=None, op0=ALU.add)
            it = lnp.tile([1, TQ], i32, tag="it")
            nc.vector.tensor_scalar(out=it[:], in0=ve.bitcast(i32),
                                    scalar1=nri_s[:, 0:1], scalar2=None,
                                    op0=ALU.logical_shift_right)
            nc.vector.tensor_scalar(out=it[:], in0=it[:], scalar1=nri_s[:, 1:2],
                                    scalar2=None, op0=ALU.bitwise_xor)
            nc.vector.tensor_tensor(out=it[:], in0=it[:], in1=nrm_s[:],
                                    op=ALU.add)
            rstd = it.bitcast(f32)
            nrt = lnp.tile([1, TQ], f32, tag="nrt")
            for _ in range(2):
                nc.vector.tensor_tensor(out=nrt[:], in0=rstd, in1=rstd,
                                        op=ALU.mult)
                nc.vector.tensor_tensor(out=nrt[:], in0=nrt[:], in1=ve[:],
                                        op=ALU.mult)
                nc.vector.tensor_scalar(out=nrt[:], in0=nrt[:], scalar1=-0.5,
                                        scalar2=1.5, op0=ALU.mult, op1=ALU.add)
                nc.vector.tensor_tensor(out=rstd, in0=rstd, in1=nrt[:],
                                        op=ALU.mult)
            rsB = lnp.tile([P, TQ], f32, tag="rsB")
            nc.gpsimd.partition_broadcast(rsB[:], rstd)
            for ft in range(FT):
                z = lnp.tile([P, TQ], f32, tag=f"z{ft % 3}")
                nc.vector.tensor_tensor(out=z[:], in0=src_f[:, ft, :],
                                        in1=muB[:], op=ALU.subtract)
                nc.vector.tensor_tensor(out=z[:], in0=z[:], in1=rsB[:],
                                        op=ALU.mult)
                out_cb(ft, z, g_s, be_s)

        # ---------------- phases C-F (all feature-major) ----------------
        with tc.tile_pool(name="lnp", bufs=2) as lnp, \
             tc.tile_pool(name="psA2", bufs=2, space="PSUM") as psA2, \
             tc.tile_pool(name="psSt", bufs=1, space="PSUM") as psSt:
            t1_s = acts.tile([P, FT, TQ], f32, tag="tres")
            t1b_s = acts.tile([P, FT, TQ], bf16, tag="tbb")
            st0a, st1a, feed_a, flush_a = make_stats(lnp, psSt, "sa")

            def wo_evict(mt_i, ps):
                nc.vector.tensor_tensor(out=t1_s[:, mt_i, :], in0=ps[:],
                                        in1=xres_s[:, mt_i, :], op=ALU.add)
                nc.scalar.copy(t1b_s[:, mt_i, :], t1_s[:, mt_i, :])
                feed_a(mt_i, t1b_s[:, mt_i, :])

            proj_T(woT, ctxF_s, wo_evict, psA2, w_first=w_pre)
            flush_a()

            def ln1_out(ft, z, g_s, be_s):
                nc.vector.tensor_scalar(
                    out=ln1b_s[:, ft, :], in0=z[:],
                    scalar1=g_s[:, ft:ft + 1], scalar2=be_s[:, ft:ft + 1],
                    op0=ALU.mult, op1=ALU.add)

            ln_apply(t1_s, st0a, st1a, g1_s, be1_s, lnp, ln1_out)

            def relu_evict(mt_i, ps):
                nc.vector.tensor_scalar(
                    out=hT_s[:, mt_i, :], in0=ps[:],
                    scalar1=b1_s[:, mt_i:mt_i + 1], scalar2=0.0,
                    op0=ALU.add, op1=ALU.max)

            proj_T(w1T, ln1b_s, relu_evict, psA2)

            t2_s = acts.tile([P, FT, TQ], f32, tag="tres")
            t2b_s = acts.tile([P, FT, TQ], bf16, tag="tbb")
            st0b, st1b, feed_b, flush_b = make_stats(lnp, psSt, "sb")

            def w2_evict(mt_i, ps):
                zt = lnp.tile([P, TQ], f32, tag="w2z")
                nc.vector.tensor_scalar(
                    out=zt[:], in0=ps[:],
                    scalar1=b2_s[:, mt_i:mt_i + 1], scalar2=None, op0=ALU.add)
                nc.vector.tensor_tensor(out=t2_s[:, mt_i, :], in0=zt[:],
                                        in1=ln1b_s[:, mt_i, :], op=ALU.add)
                nc.scalar.copy(t2b_s[:, mt_i, :], t2_s[:, mt_i, :])
                feed_b(mt_i, t2b_s[:, mt_i, :])

            proj_T(w2T, hT_s, w2_evict, psA2)
            flush_b()

            y_r = y.rearrange("(t p) n -> p t n", p=P)

            def ln2_out(ft, z, g_s, be_s):
                yv = lnp.tile([P, TQ], f32, tag=f"yv{ft % 3}")
                nc.vector.tensor_scalar(
                    out=yv[:], in0=z[:],
                    scalar1=g_s[:, ft:ft + 1], scalar2=be_s[:, ft:ft + 1],
                    op0=ALU.mult, op1=ALU.add)
                nc.sync.dma_start(y_r[:, ft, :], yv[:])

            ln_apply(t2_s, st0b, st1b, g2_s, be2_s, lnp, ln2_out)

    nc.compile()
    return nc


_NC_CACHE = {}


def _get_nc():
    if 'nc' not in _NC_CACHE:
        _NC_CACHE['nc'] = build_kernel()
    return _NC_CACHE['nc']


def _bf(a):
    return np.ascontiguousarray(np.asarray(a, np.float32)).astype(
        ml_dtypes.bfloat16)


def make_in_maps(x, Wq, bq, Wk, bk, Wv, bv, Wo, bo, W1, b1, W2, b2,
                 g1, be1, g2, be2):
    def pt(v):  # [H] -> [P, FT] partition-tiled
        return np.ascontiguousarray(np.asarray(v, np.float32).reshape(FT, P).T)

    scale = np.float32(1.0 / np.sqrt(HD))
    x = np.asarray(x, np.float32)
    xTf = np.ascontiguousarray(x.transpose(2, 0, 1).reshape(H, B * S))
    wqT = np.asarray(Wq, np.float32).T
    wkT = np.asarray(Wk, np.float32).T * scale
    wvT = np.asarray(Wv, np.float32).T
    bo = np.asarray(bo, np.float32)
    shared = {
        "xT": _bf(xTf),
        "woT": _bf(np.asarray(Wo, np.float32).T),
        "w1T": _bf(np.asarray(W1, np.float32).T),
        "w2T": _bf(np.asarray(W2, np.float32).T),
        "b1p": pt(b1), "b2p": pt(b2),
        "g1p": pt(g1), "be1p": pt(be1), "g2p": pt(g2), "be2p": pt(be2),
        "nri": np.array([[1, -1]], np.int32),
        "nrm": np.full((1, TQ), MAGIC, np.int32),
    }
    in_maps = []
    for c in range(NCORES):
        hs = slice(P * c, P * (c + 1))
        b, sl = c // 4, (c % 4) * TQ
        m = dict(shared)
        m["wqTs"] = _bf(wqT[:, hs])
        m["wkTs"] = _bf(wkT[:, hs])
        m["wvTs"] = _bf(wvT[:, hs])
        m["bqp"] = np.ascontiguousarray(
            np.asarray(bq, np.float32)[hs].reshape(P, 1))
        m["bkp"] = np.ascontiguousarray(
            (np.asarray(bk, np.float32) * scale)[hs].reshape(P, 1))
        m["bvB"] = np.ascontiguousarray(np.broadcast_to(
            np.asarray(bv, np.float32)[hs], (P, P)))
        m["xres"] = np.ascontiguousarray(x[b, sl:sl + TQ, :].T + bo[:, None])
        in_maps.append(m)
    return in_maps


def kernel(x, Wq, bq, Wk, bk, Wv, bv, Wo, bo, W1, b1, W2, b2,
           g1, be1, g2, be2):
    x = np.asarray(x)
    nc = _get_nc()
    in_maps = make_in_maps(x, Wq, bq, Wk, bk, Wv, bv, Wo, bo,
                           W1, b1, W2, b2, g1, be1, g2, be2)
    res = run_bass_kernel_spmd(nc, in_maps, list(range(NCORES)))
    out = np.empty((B, S, H), np.float32)
    for c in range(NCORES):
        b, sl = c // 4, (c % 4) * TQ
        out[b, sl:sl + TQ, :] = np.asarray(res.results[c]["y"]).T
    return out
